# revision 6
# baseline (speedup 1.0000x reference)
"""Sequence-parallel causal attention for Trainium2, 8 NeuronCores (SPMD).

Problem: o = softmax(causal((q@w_q)(k@w_k)^T/sqrt(dk))) @ (v@w_v) @ w_o
Shapes: q/k/v [8192,1024] f32, w_q/w_k/w_v [1024,128], w_o [128,1024].

Strategy (uniform-SPMD, one program for all 8 cores):
- Keys sharded: core c projects keys [1024c, 1024c+1024) -> kp^T (fp16 hi/lo
  split) and vp (fp16), AllGathered to every core.
- Rows sharded in 128-row blocks, one block per "octave" of 8: core c owns
  block gb = 8*oct + pos, pos = c (oct even) else 7-c.  Every core processes
  keys [0, 1024*(oct+1)) for its oct-block => identical instruction stream;
  the causal boundary is applied via host-built additive mask strips.
- QK path in fp16 hi/lo 3-product split (near-fp32 accuracy); softmax max
  via a bf16-rate stats pass in s-layout (row-max is only needed to ~+-70
  of true max since exp/sum renormalizes); main pass in sT layout so that
  P^T feeds the AV matmul directly; -m folded into the PSUM accumulation as
  a rank-1 (K=1) matmul; 1/sqrt(dk) folded into the exp activation scale;
  1/rowsum applied after the output projection as a per-partition scale.
"""

import math
import numpy as np

N_CORES = 8
S, D, DK = 8192, 1024, 128
RPC = S // N_CORES          # rows per core (1024)
KPC = S // N_CORES          # keys per core (1024)
NOCT = 8                    # octaves (128-row blocks per core)
NEG_BIG = -2.0e9
INV_SQRT_DK = 1.0 / math.sqrt(DK)

_CACHE = {}


def _build():
    import concourse.bass as bass
    import concourse.mybir as mybir
    import concourse.tile as tile
    from concourse import bacc
    from contextlib import ExitStack

    dt = mybir.dt
    f32, f16, f32r = dt.float32, dt.float16, dt.float32r

    nc = bacc.Bacc("TRN2", target_bir_lowering=False, debug=False,
                   num_devices=N_CORES)

    # ---- I/O ----
    q_sh = nc.declare_dram_parameter("q_sh", [RPC, D], f32, isOutput=False)
    k_sh = nc.declare_dram_parameter("k_sh", [KPC, D], f32, isOutput=False)
    v_sh = nc.declare_dram_parameter("v_sh", [KPC, D], f32, isOutput=False)
    wq_h = nc.declare_dram_parameter("wq_h", [D, DK], f16, isOutput=False)
    wq_l = nc.declare_dram_parameter("wq_l", [D, DK], f16, isOutput=False)
    wk_h = nc.declare_dram_parameter("wk_h", [D, DK], f16, isOutput=False)
    wk_l = nc.declare_dram_parameter("wk_l", [D, DK], f16, isOutput=False)
    wv_h = nc.declare_dram_parameter("wv_h", [D, DK], f16, isOutput=False)
    wv_l = nc.declare_dram_parameter("wv_l", [D, DK], f16, isOutput=False)
    wo_p = nc.declare_dram_parameter("wo", [DK, D], f32r, isOutput=False)
    maskT_p = nc.declare_dram_parameter("maskT", [2, 128, 1024], f32, isOutput=False)
    mask2_p = nc.declare_dram_parameter("mask2", [2, 128, 1024], f32, isOutput=False)
    ident_p = nc.declare_dram_parameter("ident", [128, 128], f32, isOutput=False)
    nones_p = nc.declare_dram_parameter("negones", [1, 128], f16, isOutput=False)
    ones_p = nc.declare_dram_parameter("ones_col", [128, 1], f16, isOutput=False)
    o_sh = nc.declare_dram_parameter("o_sh", [RPC, D], f32, isOutput=True)

    # ---- internal DRAM ----
    qh_d = nc.dram_tensor("qh_d", [RPC, D], f16)
    ql_d = nc.dram_tensor("ql_d", [RPC, D], f16)
    kh_d = nc.dram_tensor("kh_d", [KPC, D], f16)
    kl_d = nc.dram_tensor("kl_d", [KPC, D], f16)
    vh_d = nc.dram_tensor("vh_d", [KPC, D], f16)
    agk_in = nc.dram_tensor("agk_in", [2, 128, KPC], f16)
    agk_out = nc.dram_tensor("agk_out", [N_CORES, 2, 128, KPC], f16,
                             addr_space="Shared")
    agv_in = nc.dram_tensor("agv_in", [KPC, DK], f16)
    agv_out = nc.dram_tensor("agv_out", [N_CORES, KPC, DK], f16,
                             addr_space="Shared")

    rgroups = [list(range(N_CORES))]

    with tile.TileContext(nc) as tc, ExitStack() as ctx:
        # ---------- persistent SBUF pools ----------
        consts = ctx.enter_context(tc.tile_pool(name="consts", bufs=1))
        persist = ctx.enter_context(tc.tile_pool(name="persist", bufs=1))

        wo_sb = consts.tile([128, D], f32r, tag="wo")
        nc.sync.dma_start(wo_sb[:], wo_p[:])
        ident = consts.tile([128, 128], f32, tag="ident")
        nc.sync.dma_start(ident[:], ident_p[:])
        negones = consts.tile([1, 128], f16, tag="negones")
        nc.sync.dma_start(negones[:], nones_p[:])
        ones_col = consts.tile([128, 1], f16, tag="ones")
        nc.sync.dma_start(ones_col[:], ones_p[:])
        maskT_sb = [consts.tile([128, 1024], f32, tag=f"maskT{p}", name=f"maskT{p}") for p in range(2)]
        mask2_sb = [consts.tile([128, 1024], f32, tag=f"mask2{p}", name=f"mask2{p}") for p in range(2)]
        for p in range(2):
            nc.sync.dma_start(maskT_sb[p][:], maskT_p[p])
            nc.sync.dma_start(mask2_sb[p][:], mask2_p[p])

        # weight hi/lo tiles, SBUF layout [128, (di,dk)]
        def load_w(name, param):
            t = consts.tile([128, 8 * DK], f16, tag=name, name=name)
            nc.sync.dma_start(
                t[:].rearrange("p (di dk) -> p di dk", di=8),
                param[:].rearrange("(di p) dk -> p di dk", p=128))
            return t
        wqh_sb = load_w("wqh", wq_h); wql_sb = load_w("wql", wq_l)
        wkh_sb = load_w("wkh", wk_h); wkl_sb = load_w("wkl", wk_l)
        wvh_sb = load_w("wvh", wv_h); wvl_sb = load_w("wvl", wv_l)

        # persistent activation tiles
        qpTh = persist.tile([128, RPC], f16, tag="qpTh")
        qpTl = persist.tile([128, RPC], f16, tag="qpTl")
        kpTh = persist.tile([128, S], f16, tag="kpTh")
        kpTl = persist.tile([128, S], f16, tag="kpTl")
        vp_sb = [persist.tile([128, 8 * DK], f16, tag=f"vp{g}", name=f"vp{g}") for g in range(8)]

        # ---------- phase A/B: split, transpose, project, gather ----------
        with tc.tile_pool(name="xnat", bufs=3) as xnat, \
             tc.tile_pool(name="xsplit", bufs=4) as xsplit, \
             tc.tile_pool(name="xT", bufs=16) as xTp, \
             tc.tile_pool(name="projps", bufs=2, space="PSUM") as projps, \
             tc.tile_pool(name="projsb", bufs=4) as projsb:

            def split_tensor(xin, hd, ld):
                # natural load 128-row groups; h/l fp16 split; store to DRAM
                for i in range(8):
                    xt = xnat.tile([128, D], f32, tag="xnat")
                    nc.sync.dma_start(xt[:], xin[128 * i:128 * (i + 1), :])
                    xh = xsplit.tile([128, D], f16, tag="xh")
                    nc.scalar.copy(xh[:], xt[:])
                    nc.sync.dma_start(hd[128 * i:128 * (i + 1), :], xh[:])
                    if ld is not None:
                        xl = xsplit.tile([128, D], f16, tag="xl")
                        nc.gpsimd.tensor_sub(xl[:], xt[:], xh[:])
                        nc.sync.dma_start(ld[128 * i:128 * (i + 1), :], xl[:])

            def load_T(dram, di):
                t = xTp.tile([128, 1024], f16, tag="xT", name="xT")
                nc.sync.dma_start_transpose(t[:], dram[:, 128 * di:128 * (di + 1)])
                return t

            # ---- k ----
            split_tensor(k_sh, kh_d, kl_d)
            kTh = [load_T(kh_d, di) for di in range(8)]
            kTl = [load_T(kl_d, di) for di in range(8)]
            for half in range(2):
                ps = projps.tile([128, 512], f32, tag="projps")
                sl = slice(512 * half, 512 * (half + 1))
                n = 0
                for wt, xt in ((wkh_sb, kTh), (wkh_sb, kTl), (wkl_sb, kTh)):
                    for di in range(8):
                        nc.tensor.matmul(ps[:], wt[:, 128 * di:128 * (di + 1)],
                                         xt[di][:, sl], start=(n == 0), stop=(n == 23))
                        n += 1
                hh = projsb.tile([128, 512], f16, tag="ph")
                nc.scalar.copy(hh[:], ps[:])
                ll = projsb.tile([128, 512], f16, tag="pl")
                nc.vector.tensor_sub(ll[:], ps[:], hh[:])
                nc.sync.dma_start(agk_in[0, :, sl], hh[:])
                nc.sync.dma_start(agk_in[1, :, sl], ll[:])
            nc.gpsimd.collective_compute(
                "AllGather", mybir.AluOpType.bypass, replica_groups=rgroups,
                ins=[agk_in[:]], outs=[agk_out[:]])

            # ---- v ----
            split_tensor(v_sh, vh_d, None)
            vTh = [load_T(vh_d, di) for di in range(8)]
            for kt in range(8):
                ps = projps.tile([128, 128], f32, tag="projpsv")
                ksl = slice(128 * kt, 128 * (kt + 1))
                n = 0
                for wt in (wvh_sb, wvl_sb):
                    for di in range(8):
                        nc.tensor.matmul(ps[:], vTh[di][:, ksl],
                                         wt[:, 128 * di:128 * (di + 1)],
                                         start=(n == 0), stop=(n == 15))
                        n += 1
                vh16 = projsb.tile([128, 128], f16, tag="vh16")
                nc.scalar.copy(vh16[:], ps[:])
                nc.sync.dma_start(
                    agv_in[ksl, :].rearrange("p dv -> p dv"), vh16[:])
            nc.gpsimd.collective_compute(
                "AllGather", mybir.AluOpType.bypass, replica_groups=rgroups,
                ins=[agv_in[:]], outs=[agv_out[:]])

            # ---- q ----
            split_tensor(q_sh, qh_d, ql_d)
            qTh = [load_T(qh_d, di) for di in range(8)]
            qTl = [load_T(ql_d, di) for di in range(8)]
            for half in range(2):
                ps = projps.tile([128, 512], f32, tag="projps")
                sl = slice(512 * half, 512 * (half + 1))
                n = 0
                for wt, xt in ((wqh_sb, qTh), (wqh_sb, qTl), (wql_sb, qTh)):
                    for di in range(8):
                        nc.tensor.matmul(ps[:], wt[:, 128 * di:128 * (di + 1)],
                                         xt[di][:, sl], start=(n == 0), stop=(n == 23))
                        n += 1
                nc.scalar.copy(qpTh[:, sl], ps[:])
                nc.vector.tensor_sub(qpTl[:, sl], ps[:], qpTh[:, sl])

            # ---- gathered loads ----
            nc.sync.dma_start(
                kpTh[:].rearrange("p (g f) -> p g f", g=8),
                agk_out[:, 0].rearrange("g p f -> p g f"))
            nc.sync.dma_start(
                kpTl[:].rearrange("p (g f) -> p g f", g=8),
                agk_out[:, 1].rearrange("g p f -> p g f"))
            for g in range(8):
                nc.sync.dma_start(
                    vp_sb[g][:].rearrange("p (rb dv) -> p rb dv", rb=8),
                    agv_out[g].rearrange("a b -> (a b)")
                    .rearrange("(rb p dv) -> p rb dv", rb=8, p=128))

        # ---------- phase C: attention ----------
        with tc.tile_pool(name="statps", bufs=2, space="PSUM") as statps, \
             tc.tile_pool(name="sTps", bufs=2, space="PSUM") as sTps, \
             tc.tile_pool(name="oTps", bufs=2, space="PSUM") as oTps, \
             tc.tile_pool(name="smps", bufs=1, space="PSUM") as smps, \
             tc.tile_pool(name="mps", bufs=1, space="PSUM") as mps, \
             tc.tile_pool(name="mpool", bufs=2) as mpool, \
             tc.tile_pool(name="pTpool", bufs=3) as pTpool, \
             tc.tile_pool(name="osb", bufs=4) as osb:

            for oct_ in range(NOCT):
                par = oct_ % 2
                rsl = slice(128 * oct_, 128 * (oct_ + 1))
                nkt = 8 * (oct_ + 1)
                ngrp = 2 * (oct_ + 1)

                # ---- stats: row max m over keys [0, 1024*(oct+1)) ----
                m_b = mpool.tile([128, 1], f32, tag="m")
                for st in range(ngrp):
                    ps_s = statps.tile([128, 512], f32, tag="stat")
                    nc.tensor.matmul(ps_s[:], qpTh[:, rsl],
                                     kpTh[:, 512 * st:512 * (st + 1)],
                                     start=True, stop=True)
                    if st >= 2 * oct_:
                        w = st - 2 * oct_
                        nc.vector.tensor_add(
                            ps_s[:], ps_s[:],
                            mask2_sb[par][:, 512 * w:512 * (w + 1)])
                    mx = mpool.tile([128, 1], f32, tag="mx")
                    nc.vector.reduce_max(mx[:], ps_s[:], axis=mybir.AxisListType.X)
                    if st == 0:
                        nc.vector.tensor_copy(m_b[:], mx[:])
                    else:
                        nc.vector.tensor_max(m_b[:], m_b[:], mx[:])

                # m as a row vector [1,128] fp16 (for the K=1 rank-1 subtract)
                mrep_ps = mps.tile([1, 128], f32, tag="mrep")
                nc.tensor.transpose(mrep_ps[:], m_b[:], ident[:])
                # m/16 in fp16 (m can exceed fp16 max; the rank-1 lhsT is -16)
                mrep = mpool.tile([1, 128], f16, tag="mrepsb")
                nc.scalar.mul(mrep[:], mrep_ps[:], 1.0 / 16.0)

                # ---- main pass over key tiles, groups of 4 ----
                oT = oTps.tile([128, 128], f32, tag="oT")
                sm = smps.tile([128, 1], f32, tag="sm")
                for grp in range(ngrp):
                    ps_g = sTps.tile([128, 512], f32, tag="sT")
                    for t4 in range(4):
                        kt = 4 * grp + t4
                        ksl = slice(128 * kt, 128 * (kt + 1))
                        sl = ps_g[:, 128 * t4:128 * (t4 + 1)]
                        nc.tensor.matmul(sl, kpTh[:, ksl], qpTh[:, rsl],
                                         start=True, stop=False)
                        nc.tensor.matmul(sl, kpTl[:, ksl], qpTh[:, rsl],
                                         start=False, stop=False)
                        nc.tensor.matmul(sl, kpTh[:, ksl], qpTl[:, rsl],
                                         start=False, stop=False)
                        nc.tensor.matmul(sl, negones[:], mrep[:],
                                         start=False, stop=True)
                    if grp >= 2 * oct_:
                        w = grp - 2 * oct_
                        nc.vector.tensor_add(
                            ps_g[:], ps_g[:],
                            maskT_sb[par][:, 512 * w:512 * (w + 1)])
                    pT = pTpool.tile([128, 512], f16, tag="pT")
                    nc.scalar.activation(pT[:], ps_g[:],
                                         mybir.ActivationFunctionType.Exp,
                                         scale=INV_SQRT_DK)
                    for t4 in range(4):
                        kt = 4 * grp + t4
                        g, rb = kt // 8, kt % 8
                        psl = pT[:, 128 * t4:128 * (t4 + 1)]
                        nc.tensor.matmul(oT[:],
                                         vp_sb[g][:, 128 * rb:128 * (rb + 1)], psl,
                                         start=(kt == 0), stop=(kt == nkt - 1))
                        nc.tensor.matmul(sm[:], psl, ones_col[:],
                                         start=(kt == 0), stop=(kt == nkt - 1))

                # ---- normalize + output projection ----
                oT_sb = osb.tile([128, 128], f32r, tag="oTsb")
                nc.vector.tensor_copy(oT_sb[:], oT[:])
                rsum = mpool.tile([128, 1], f32, tag="rsum")
                nc.vector.reciprocal(rsum[:], sm[:])
                for half in range(2):
                    sl = slice(512 * half, 512 * (half + 1))
                    ps_o = sTps.tile([128, 512], f32, tag="sT")
                    nc.tensor.matmul(ps_o[:], oT_sb[:], wo_sb[:, sl],
                                     start=True, stop=True)
                    out_sb = osb.tile([128, 512], f32, tag="outsb")
                    nc.scalar.activation(out_sb[:], ps_o[:],
                                         mybir.ActivationFunctionType.Copy,
                                         scale=rsum[:])
                    nc.sync.dma_start(o_sh[rsl, sl], out_sb[:])

    nc.compile()
    return nc


def _host_inputs(q, k, v, w_q, w_k, w_v, w_o):
    """Build per-core input maps (host-side sharding)."""
    f16 = np.float16
    # weight hi/lo splits
    def split(w):
        h = w.astype(f16)
        l = (w - h.astype(np.float32)).astype(f16)
        return h, l
    wq_h, wq_l = split(w_q)
    wk_h, wk_l = split(w_k)
    wv_h, wv_l = split(w_v)

    ident = np.eye(128, dtype=np.float32)
    negones = np.full((1, 128), -16.0, dtype=f16)
    ones_col = np.ones((128, 1), dtype=f16)

    kidx = np.arange(128)[:, None]
    t_f = np.arange(1024)[None, :] // 128
    r_f = np.arange(1024)[None, :] % 128
    ridx = np.arange(128)[:, None]
    kk_f = np.arange(1024)[None, :]

    in_maps = []
    for c in range(N_CORES):
        blocks = [8 * o + (c if o % 2 == 0 else 7 - c) for o in range(NOCT)]
        q_rows = np.concatenate([q[128 * gb:128 * (gb + 1)] for gb in blocks])
        maskT = np.empty((2, 128, 1024), np.float32)
        mask2 = np.empty((2, 128, 1024), np.float32)
        for p, pos in enumerate((c, 7 - c)):
            maskT[p] = np.where(128 * t_f + kidx <= 128 * pos + r_f, 0.0, NEG_BIG)
            mask2[p] = np.where(kk_f <= 128 * pos + ridx, 0.0, NEG_BIG)
        in_maps.append({
            "q_sh": np.ascontiguousarray(q_rows),
            "k_sh": np.ascontiguousarray(k[KPC * c:KPC * (c + 1)]),
            "v_sh": np.ascontiguousarray(v[KPC * c:KPC * (c + 1)]),
            "wq_h": wq_h, "wq_l": wq_l, "wk_h": wk_h, "wk_l": wk_l,
            "wv_h": wv_h, "wv_l": wv_l, "wo": w_o,
            "maskT": maskT, "mask2": mask2, "ident": ident,
            "negones": negones, "ones_col": ones_col,
        })
    return in_maps


def kernel(q, k, v, w_q, w_k, w_v, w_o):
    from concourse.bass_utils import run_bass_kernel_spmd

    q = np.asarray(q, dtype=np.float32)
    k = np.asarray(k, dtype=np.float32)
    v = np.asarray(v, dtype=np.float32)
    w_q = np.asarray(w_q, dtype=np.float32)
    w_k = np.asarray(w_k, dtype=np.float32)
    w_v = np.asarray(w_v, dtype=np.float32)
    w_o = np.asarray(w_o, dtype=np.float32)

    if "nc" not in _CACHE:
        _CACHE["nc"] = _build()
    nc = _CACHE["nc"]

    in_maps = _host_inputs(q, k, v, w_q, w_k, w_v, w_o)
    res = run_bass_kernel_spmd(nc, in_maps, list(range(N_CORES)))

    out = np.empty((S, D), dtype=np.float32)
    for c in range(N_CORES):
        o_sh = res.results[c]["o_sh"]
        for o in range(NOCT):
            gb = 8 * o + (c if o % 2 == 0 else 7 - c)
            out[128 * gb:128 * (gb + 1)] = o_sh[128 * o:128 * (o + 1)]
    return out


# revision 13
# speedup vs baseline: 1.0304x; 1.0304x over previous
"""Sequence-parallel causal attention for Trainium2, 8 NeuronCores (SPMD).

Problem: o = softmax(causal((q@w_q)(k@w_k)^T/sqrt(dk))) @ (v@w_v) @ w_o
Shapes: q/k/v [8192,1024] f32, w_q/w_k/w_v [1024,128], w_o [128,1024].

Strategy (uniform-SPMD, one program for all 8 cores):
- Keys sharded: core c projects keys [1024c, 1024c+1024) -> kp^T (fp16 hi/lo
  split) and vp (fp16), AllGathered to every core.
- Rows sharded in 128-row blocks, one block per "octave" of 8: core c owns
  block gb = 8*oct + pos, pos = c (oct even) else 7-c.  Every core processes
  keys [0, 1024*(oct+1)) for its oct-block => identical instruction stream;
  the causal boundary is applied via host-built additive mask strips.
- QK path in fp16 hi/lo 3-product split (near-fp32 accuracy); softmax max
  via a bf16-rate stats pass in s-layout (row-max is only needed to ~+-70
  of true max since exp/sum renormalizes); main pass in sT layout so that
  P^T feeds the AV matmul directly; -m folded into the PSUM accumulation as
  a rank-1 (K=1) matmul; 1/sqrt(dk) folded into the exp activation scale;
  1/rowsum applied after the output projection as a per-partition scale.
"""

import math
import numpy as np

N_CORES = 8
S, D, DK = 8192, 1024, 128
RPC = S // N_CORES          # rows per core (1024)
KPC = S // N_CORES          # keys per core (1024)
NOCT = 8                    # octaves (128-row blocks per core)
NEG_BIG = -2.0e9
INV_SQRT_DK = 1.0 / math.sqrt(DK)

_CACHE = {}


def _build():
    import concourse.bass as bass
    import concourse.mybir as mybir
    import concourse.tile as tile
    from concourse import bacc
    from contextlib import ExitStack

    dt = mybir.dt
    f32, f16, f32r = dt.float32, dt.float16, dt.float32r

    nc = bacc.Bacc("TRN2", target_bir_lowering=False, debug=False,
                   num_devices=N_CORES)

    # ---- I/O ----
    q_sh = nc.declare_dram_parameter("q_sh", [RPC, D], f32, isOutput=False)
    k_sh = nc.declare_dram_parameter("k_sh", [KPC, D], f32, isOutput=False)
    v_sh = nc.declare_dram_parameter("v_sh", [KPC, D], f32, isOutput=False)
    wq_h = nc.declare_dram_parameter("wq_h", [D, DK], f16, isOutput=False)
    wq_l = nc.declare_dram_parameter("wq_l", [D, DK], f16, isOutput=False)
    wk_h = nc.declare_dram_parameter("wk_h", [D, DK], f16, isOutput=False)
    wk_l = nc.declare_dram_parameter("wk_l", [D, DK], f16, isOutput=False)
    wv_h = nc.declare_dram_parameter("wv_h", [D, DK], f16, isOutput=False)
    wv_l = nc.declare_dram_parameter("wv_l", [D, DK], f16, isOutput=False)
    wo_p = nc.declare_dram_parameter("wo", [DK, D], f32r, isOutput=False)
    maskT_p = nc.declare_dram_parameter("maskT", [2, 128, 1024], f32, isOutput=False)
    mask2_p = nc.declare_dram_parameter("mask2", [2, 128, 1024], f32, isOutput=False)
    ident_p = nc.declare_dram_parameter("ident", [128, 128], f32, isOutput=False)
    ident16_p = nc.declare_dram_parameter("ident16", [128, 128], f16, isOutput=False)
    nones_p = nc.declare_dram_parameter("negones", [1, 128], f16, isOutput=False)
    ones_p = nc.declare_dram_parameter("ones_col", [128, 1], f16, isOutput=False)
    o_sh = nc.declare_dram_parameter("o_sh", [RPC, D], f32, isOutput=True)

    # ---- internal DRAM ----
    agk_in = nc.dram_tensor("agk_in", [2, 128, KPC], f16)
    agk_out = nc.dram_tensor("agk_out", [N_CORES, 2, 128, KPC], f16,
                             addr_space="Shared")
    agv_in = nc.dram_tensor("agv_in", [KPC, DK], f16)
    agv_out = nc.dram_tensor("agv_out", [N_CORES, KPC, DK], f16,
                             addr_space="Shared")

    rgroups = [list(range(N_CORES))]

    with tile.TileContext(nc) as tc, ExitStack() as ctx:
        # ---------- persistent SBUF pools ----------
        consts = ctx.enter_context(tc.tile_pool(name="consts", bufs=1))
        persist = ctx.enter_context(tc.tile_pool(name="persist", bufs=1))

        wo_sb = consts.tile([128, D], f32r, tag="wo")
        nc.sync.dma_start(wo_sb[:], wo_p[:])
        ident = consts.tile([128, 128], f32, tag="ident")
        nc.sync.dma_start(ident[:], ident_p[:])
        ident16 = consts.tile([128, 128], f16, tag="ident16")
        nc.sync.dma_start(ident16[:], ident16_p[:])
        negones = consts.tile([1, 128], f16, tag="negones")
        nc.sync.dma_start(negones[:], nones_p[:])
        ones_col = consts.tile([128, 1], f16, tag="ones")
        nc.sync.dma_start(ones_col[:], ones_p[:])
        maskT_sb = [consts.tile([128, 1024], f32, tag=f"maskT{p}", name=f"maskT{p}") for p in range(2)]
        mask2_sb = [consts.tile([128, 1024], f32, tag=f"mask2{p}", name=f"mask2{p}") for p in range(2)]
        for p in range(2):
            nc.sync.dma_start(maskT_sb[p][:], maskT_p[p])
            nc.sync.dma_start(mask2_sb[p][:], mask2_p[p])

        # weight hi/lo tiles, SBUF layout [128, (di,dk)]
        def load_w(name, param):
            t = consts.tile([128, 8 * DK], f16, tag=name, name=name)
            nc.sync.dma_start(
                t[:].rearrange("p (di dk) -> p di dk", di=8),
                param[:].rearrange("(di p) dk -> p di dk", p=128))
            return t
        wqh_sb = load_w("wqh", wq_h); wql_sb = load_w("wql", wq_l)
        wkh_sb = load_w("wkh", wk_h); wkl_sb = load_w("wkl", wk_l)
        wvh_sb = load_w("wvh", wv_h); wvl_sb = load_w("wvl", wv_l)

        # persistent activation tiles
        qpTh = persist.tile([128, RPC], f16, tag="qpTh")
        qpTl = persist.tile([128, RPC], f16, tag="qpTl")
        kpTh = persist.tile([128, S], f16, tag="kpTh")
        kpTl = persist.tile([128, S], f16, tag="kpTl")
        vp_sb = [persist.tile([128, 8 * DK], f16, tag=f"vp{g}", name=f"vp{g}") for g in range(8)]

        # ---------- phase A/B: split, transpose, project, gather ----------
        with tc.tile_pool(name="xnat", bufs=3) as xnat, \
             tc.tile_pool(name="xsplit", bufs=4) as xsplit, \
             tc.tile_pool(name="xT", bufs=26) as xTp, \
             tc.tile_pool(name="projps", bufs=2, space="PSUM") as projps, \
             tc.tile_pool(name="tps", bufs=4, space="PSUM") as tps, \
             tc.tile_pool(name="projsb", bufs=4) as projsb:

            def split_tensor(xin, with_l):
                # batched natural loads (256 rows/DMA), fp16 h/l split,
                # PE-transpose [128,128] blocks into [D-sub, rows] tiles.
                xTh = [xTp.tile([128, KPC], f16, tag="xT", name="xT")
                       for _ in range(8)]
                xTl = [xTp.tile([128, KPC], f16, tag="xT", name="xT")
                       for _ in range(8)] if with_l else None
                for i2 in range(4):
                    xt = xnat.tile([128, 2 * D], f32, tag="xnat")
                    nc.sync.dma_start(
                        xt[:].rearrange("p (g d) -> p g d", g=2),
                        xin[256 * i2:256 * (i2 + 1), :]
                        .rearrange("(g p) d -> p g d", p=128))
                    xh = xsplit.tile([128, 2 * D], f16, tag="xh")
                    nc.vector.tensor_copy(xh[:], xt[:])
                    xl = None
                    if with_l:
                        xl = xsplit.tile([128, 2 * D], f16, tag="xl")
                        nc.gpsimd.tensor_sub(xl[:], xt[:], xh[:])
                    rs2 = slice(256 * i2, 256 * (i2 + 1))
                    for di in range(8):
                        pt = tps.tile([128, 256], f16, tag="tps", name="tps")
                        for g in range(2):
                            dsl = slice(1024 * g + 128 * di,
                                        1024 * g + 128 * (di + 1))
                            nc.tensor.transpose(pt[:, 128 * g:128 * (g + 1)],
                                                xh[:, dsl], ident16[:])
                        nc.scalar.copy(xTh[di][:, rs2], pt[:])
                        if with_l:
                            pt2 = tps.tile([128, 256], f16, tag="tps",
                                           name="tps2")
                            for g in range(2):
                                dsl = slice(1024 * g + 128 * di,
                                            1024 * g + 128 * (di + 1))
                                nc.tensor.transpose(
                                    pt2[:, 128 * g:128 * (g + 1)],
                                    xl[:, dsl], ident16[:])
                            nc.vector.tensor_copy(xTl[di][:, rs2], pt2[:])
                return xTh, xTl

            # ---- k ----
            kTh, kTl = split_tensor(k_sh, True)
            for half in range(2):
                ps = projps.tile([128, 512], f32, tag="projps")
                sl = slice(512 * half, 512 * (half + 1))
                n = 0
                for wt, xt in ((wkh_sb, kTh), (wkl_sb, kTh), (wkh_sb, kTl)):
                    for di in range(8):
                        nc.tensor.matmul(ps[:], wt[:, 128 * di:128 * (di + 1)],
                                         xt[di][:, sl], start=(n == 0), stop=(n == 23))
                        n += 1
                hh = projsb.tile([128, 512], f16, tag="ph")
                nc.scalar.copy(hh[:], ps[:])
                ll = projsb.tile([128, 512], f16, tag="pl")
                nc.vector.tensor_sub(ll[:], ps[:], hh[:])
                nc.sync.dma_start(agk_in[0, :, sl], hh[:])
                nc.sync.dma_start(agk_in[1, :, sl], ll[:])
            nc.gpsimd.collective_compute(
                "AllGather", mybir.AluOpType.bypass, replica_groups=rgroups,
                ins=[agk_in[:]], outs=[agk_out[:]])

            # ---- v ----
            vTh, _ = split_tensor(v_sh, False)
            for kt in range(8):
                ps = projps.tile([128, 128], f32, tag="projpsv")
                ksl = slice(128 * kt, 128 * (kt + 1))
                n = 0
                for wt in (wvh_sb, wvl_sb):
                    for di in range(8):
                        nc.tensor.matmul(ps[:], vTh[di][:, ksl],
                                         wt[:, 128 * di:128 * (di + 1)],
                                         start=(n == 0), stop=(n == 15))
                        n += 1
                vh16 = projsb.tile([128, 128], f16, tag="vh16")
                nc.scalar.copy(vh16[:], ps[:])
                nc.sync.dma_start(
                    agv_in[ksl, :].rearrange("p dv -> p dv"), vh16[:])
            nc.gpsimd.collective_compute(
                "AllGather", mybir.AluOpType.bypass, replica_groups=rgroups,
                ins=[agv_in[:]], outs=[agv_out[:]])

            # ---- q ----
            qTh, qTl = split_tensor(q_sh, True)
            for half in range(2):
                ps = projps.tile([128, 512], f32, tag="projps")
                sl = slice(512 * half, 512 * (half + 1))
                n = 0
                for wt, xt in ((wqh_sb, qTh), (wql_sb, qTh), (wqh_sb, qTl)):
                    for di in range(8):
                        nc.tensor.matmul(ps[:], wt[:, 128 * di:128 * (di + 1)],
                                         xt[di][:, sl], start=(n == 0), stop=(n == 23))
                        n += 1
                nc.scalar.copy(qpTh[:, sl], ps[:])
                nc.vector.tensor_sub(qpTl[:, sl], ps[:], qpTh[:, sl])

            # ---- gathered loads ----
            nc.sync.dma_start(
                kpTh[:].rearrange("p (g f) -> p g f", g=8),
                agk_out[:, 0].rearrange("g p f -> p g f"))
            nc.sync.dma_start(
                kpTl[:].rearrange("p (g f) -> p g f", g=8),
                agk_out[:, 1].rearrange("g p f -> p g f"))
            for g in range(8):
                nc.sync.dma_start(
                    vp_sb[g][:].rearrange("p (rb dv) -> p rb dv", rb=8),
                    agv_out[g].rearrange("a b -> (a b)")
                    .rearrange("(rb p dv) -> p rb dv", rb=8, p=128))

        # ---------- phase C: attention ----------
        with tc.tile_pool(name="statps", bufs=2, space="PSUM") as statps, \
             tc.tile_pool(name="sTps", bufs=2, space="PSUM") as sTps, \
             tc.tile_pool(name="oTps", bufs=2, space="PSUM") as oTps, \
             tc.tile_pool(name="smps", bufs=1, space="PSUM") as smps, \
             tc.tile_pool(name="mps", bufs=1, space="PSUM") as mps, \
             tc.tile_pool(name="mpool", bufs=2) as mpool, \
             tc.tile_pool(name="pTpool", bufs=3) as pTpool, \
             tc.tile_pool(name="osb", bufs=4) as osb:

            for oct_ in range(NOCT):
                par = oct_ % 2
                rsl = slice(128 * oct_, 128 * (oct_ + 1))
                nkt = 8 * (oct_ + 1)
                ngrp = 2 * (oct_ + 1)

                # ---- stats: row max m over keys [0, 1024*(oct+1)) ----
                m_b = mpool.tile([128, 1], f32, tag="m")
                for st in range(ngrp):
                    ps_s = statps.tile([128, 512], f32, tag="stat")
                    nc.tensor.matmul(ps_s[:], qpTh[:, rsl],
                                     kpTh[:, 512 * st:512 * (st + 1)],
                                     start=True, stop=True)
                    if st >= 2 * oct_:
                        w = st - 2 * oct_
                        nc.vector.tensor_add(
                            ps_s[:], ps_s[:],
                            mask2_sb[par][:, 512 * w:512 * (w + 1)])
                    mx = mpool.tile([128, 1], f32, tag="mx")
                    nc.vector.reduce_max(mx[:], ps_s[:], axis=mybir.AxisListType.X)
                    if st == 0:
                        nc.vector.tensor_copy(m_b[:], mx[:])
                    else:
                        nc.vector.tensor_max(m_b[:], m_b[:], mx[:])

                # m as a row vector [1,128] fp16 (for the K=1 rank-1 subtract)
                mrep_ps = mps.tile([1, 128], f32, tag="mrep")
                nc.tensor.transpose(mrep_ps[:], m_b[:], ident[:])
                # m/16 in fp16 (m can exceed fp16 max; the rank-1 lhsT is -16)
                mrep = mpool.tile([1, 128], f16, tag="mrepsb")
                nc.scalar.mul(mrep[:], mrep_ps[:], 1.0 / 16.0)

                # ---- main pass over key tiles, groups of 4 ----
                oT = oTps.tile([128, 128], f32, tag="oT")
                sm = smps.tile([128, 1], f32, tag="sm")
                for grp in range(ngrp):
                    ps_g = sTps.tile([128, 512], f32, tag="sT")
                    for t4 in range(4):
                        kt = 4 * grp + t4
                        ksl = slice(128 * kt, 128 * (kt + 1))
                        sl = ps_g[:, 128 * t4:128 * (t4 + 1)]
                        nc.tensor.matmul(sl, kpTh[:, ksl], qpTh[:, rsl],
                                         start=True, stop=False)
                        nc.tensor.matmul(sl, kpTh[:, ksl], qpTl[:, rsl],
                                         start=False, stop=False)
                        nc.tensor.matmul(sl, kpTl[:, ksl], qpTh[:, rsl],
                                         start=False, stop=False)
                        nc.tensor.matmul(sl, negones[:], mrep[:],
                                         start=False, stop=True)
                    if grp >= 2 * oct_:
                        w = grp - 2 * oct_
                        nc.vector.tensor_add(
                            ps_g[:], ps_g[:],
                            maskT_sb[par][:, 512 * w:512 * (w + 1)])
                    pT = pTpool.tile([128, 512], f16, tag="pT")
                    nc.scalar.activation(pT[:], ps_g[:],
                                         mybir.ActivationFunctionType.Exp,
                                         scale=INV_SQRT_DK)
                    for t4 in range(4):
                        kt = 4 * grp + t4
                        g, rb = kt // 8, kt % 8
                        psl = pT[:, 128 * t4:128 * (t4 + 1)]
                        nc.tensor.matmul(oT[:],
                                         vp_sb[g][:, 128 * rb:128 * (rb + 1)], psl,
                                         start=(kt == 0), stop=(kt == nkt - 1))
                        nc.tensor.matmul(sm[:], psl, ones_col[:],
                                         start=(kt == 0), stop=(kt == nkt - 1))

                # ---- normalize + output projection ----
                oT_sb = osb.tile([128, 128], f32r, tag="oTsb")
                nc.vector.tensor_copy(oT_sb[:], oT[:])
                rsum = mpool.tile([128, 1], f32, tag="rsum")
                nc.vector.reciprocal(rsum[:], sm[:])
                out_full = osb.tile([128, 1024], f32, tag="outfull",
                                    name="outfull")
                for half in range(2):
                    sl = slice(512 * half, 512 * (half + 1))
                    ps_o = sTps.tile([128, 512], f32, tag="sT")
                    nc.tensor.matmul(ps_o[:], oT_sb[:], wo_sb[:, sl],
                                     start=True, stop=True)
                    nc.scalar.activation(out_full[:, sl], ps_o[:],
                                         mybir.ActivationFunctionType.Copy,
                                         scale=rsum[:])
                nc.sync.dma_start(o_sh[rsl, :], out_full[:])

    nc.compile()
    return nc


def _host_inputs(q, k, v, w_q, w_k, w_v, w_o):
    """Build per-core input maps (host-side sharding)."""
    f16 = np.float16
    # weight hi/lo splits
    def split(w):
        h = w.astype(f16)
        l = (w - h.astype(np.float32)).astype(f16)
        return h, l
    wq_h, wq_l = split(w_q)
    wk_h, wk_l = split(w_k)
    wv_h, wv_l = split(w_v)

    ident = np.eye(128, dtype=np.float32)
    ident16 = np.eye(128, dtype=f16)
    negones = np.full((1, 128), -16.0, dtype=f16)
    ones_col = np.ones((128, 1), dtype=f16)

    kidx = np.arange(128)[:, None]
    t_f = np.arange(1024)[None, :] // 128
    r_f = np.arange(1024)[None, :] % 128
    ridx = np.arange(128)[:, None]
    kk_f = np.arange(1024)[None, :]

    in_maps = []
    for c in range(N_CORES):
        blocks = [8 * o + (c if o % 2 == 0 else 7 - c) for o in range(NOCT)]
        q_rows = np.concatenate([q[128 * gb:128 * (gb + 1)] for gb in blocks])
        maskT = np.empty((2, 128, 1024), np.float32)
        mask2 = np.empty((2, 128, 1024), np.float32)
        for p, pos in enumerate((c, 7 - c)):
            maskT[p] = np.where(128 * t_f + kidx <= 128 * pos + r_f, 0.0, NEG_BIG)
            mask2[p] = np.where(kk_f <= 128 * pos + ridx, 0.0, NEG_BIG)
        in_maps.append({
            "q_sh": np.ascontiguousarray(q_rows),
            "k_sh": np.ascontiguousarray(k[KPC * c:KPC * (c + 1)]),
            "v_sh": np.ascontiguousarray(v[KPC * c:KPC * (c + 1)]),
            "wq_h": wq_h, "wq_l": wq_l, "wk_h": wk_h, "wk_l": wk_l,
            "wv_h": wv_h, "wv_l": wv_l, "wo": w_o,
            "maskT": maskT, "mask2": mask2, "ident": ident, "ident16": ident16,
            "negones": negones, "ones_col": ones_col,
        })
    return in_maps


def kernel(q, k, v, w_q, w_k, w_v, w_o):
    from concourse.bass_utils import run_bass_kernel_spmd

    q = np.asarray(q, dtype=np.float32)
    k = np.asarray(k, dtype=np.float32)
    v = np.asarray(v, dtype=np.float32)
    w_q = np.asarray(w_q, dtype=np.float32)
    w_k = np.asarray(w_k, dtype=np.float32)
    w_v = np.asarray(w_v, dtype=np.float32)
    w_o = np.asarray(w_o, dtype=np.float32)

    if "nc" not in _CACHE:
        _CACHE["nc"] = _build()
    nc = _CACHE["nc"]

    in_maps = _host_inputs(q, k, v, w_q, w_k, w_v, w_o)
    res = run_bass_kernel_spmd(nc, in_maps, list(range(N_CORES)))

    out = np.empty((S, D), dtype=np.float32)
    for c in range(N_CORES):
        o_sh = res.results[c]["o_sh"]
        for o in range(NOCT):
            gb = 8 * o + (c if o % 2 == 0 else 7 - c)
            out[128 * gb:128 * (gb + 1)] = o_sh[128 * o:128 * (o + 1)]
    return out


# revision 15
# speedup vs baseline: 16608.0826x; 16118.7402x over previous
"""Sequence-parallel causal attention for Trainium2, 8 NeuronCores (SPMD).

Problem: o = softmax(causal((q@w_q)(k@w_k)^T/sqrt(dk))) @ (v@w_v) @ w_o
Shapes: q/k/v [8192,1024] f32, w_q/w_k/w_v [1024,128], w_o [128,1024].

Strategy (uniform-SPMD, one program for all 8 cores):
- Keys sharded: core c projects keys [1024c, 1024c+1024) -> kp^T (fp16 hi/lo
  split) and vp (fp16), AllGathered to every core.
- Rows sharded in 128-row blocks, one block per "octave" of 8: core c owns
  block gb = 8*oct + pos, pos = c (oct even) else 7-c.  Every core processes
  keys [0, 1024*(oct+1)) for its oct-block => identical instruction stream;
  the causal boundary is applied via host-built additive mask strips.
- QK path in fp16 hi/lo 3-product split (near-fp32 accuracy); softmax max
  via a bf16-rate stats pass in s-layout (row-max is only needed to ~+-70
  of true max since exp/sum renormalizes); main pass in sT layout so that
  P^T feeds the AV matmul directly; -m folded into the PSUM accumulation as
  a rank-1 (K=1) matmul; 1/sqrt(dk) folded into the exp activation scale;
  1/rowsum applied after the output projection as a per-partition scale.
"""

import math
import numpy as np

N_CORES = 8
S, D, DK = 8192, 1024, 128
RPC = S // N_CORES          # rows per core (1024)
KPC = S // N_CORES          # keys per core (1024)
NOCT = 8                    # octaves (128-row blocks per core)
NEG_BIG = -2.0e9
INV_SQRT_DK = 1.0 / math.sqrt(DK)

_CACHE = {}


def _build():
    import concourse.bass as bass
    import concourse.mybir as mybir
    import concourse.tile as tile
    from concourse import bacc
    from contextlib import ExitStack

    dt = mybir.dt
    f32, f16, f32r = dt.float32, dt.float16, dt.float32r

    nc = bacc.Bacc("TRN2", target_bir_lowering=False, debug=False,
                   num_devices=N_CORES)

    # ---- I/O ----
    q_sh = nc.declare_dram_parameter("q_sh", [RPC, D], f32, isOutput=False)
    k_sh = nc.declare_dram_parameter("k_sh", [KPC, D], f32, isOutput=False)
    v_sh = nc.declare_dram_parameter("v_sh", [KPC, D], f32, isOutput=False)
    wq_h = nc.declare_dram_parameter("wq_h", [D, DK], f16, isOutput=False)
    wq_l = nc.declare_dram_parameter("wq_l", [D, DK], f16, isOutput=False)
    wk_h = nc.declare_dram_parameter("wk_h", [D, DK], f16, isOutput=False)
    wk_l = nc.declare_dram_parameter("wk_l", [D, DK], f16, isOutput=False)
    wv_h = nc.declare_dram_parameter("wv_h", [D, DK], f16, isOutput=False)
    wv_l = nc.declare_dram_parameter("wv_l", [D, DK], f16, isOutput=False)
    wo_p = nc.declare_dram_parameter("wo", [DK, D], f32r, isOutput=False)
    maskT_p = nc.declare_dram_parameter("maskT", [2, 128, 1024], f32, isOutput=False)
    mask2_p = nc.declare_dram_parameter("mask2", [2, 128, 1024], f32, isOutput=False)
    ident_p = nc.declare_dram_parameter("ident", [128, 128], f32, isOutput=False)
    ident16_p = nc.declare_dram_parameter("ident16", [128, 128], f16, isOutput=False)
    nones_p = nc.declare_dram_parameter("negones", [1, 128], f16, isOutput=False)
    ones_p = nc.declare_dram_parameter("ones_col", [128, 1], f16, isOutput=False)
    o_sh = nc.declare_dram_parameter("o_sh", [RPC, D], f32, isOutput=True)

    # ---- internal DRAM ----
    agk_in = nc.dram_tensor("agk_in", [2, 128, KPC], f16)
    agk_out = nc.dram_tensor("agk_out", [N_CORES, 2, 128, KPC], f16,
                             addr_space="Shared")
    agv_in = nc.dram_tensor("agv_in", [KPC, DK], f16)
    agv_out = nc.dram_tensor("agv_out", [N_CORES, KPC, DK], f16,
                             addr_space="Shared")

    rgroups = [list(range(N_CORES))]

    with tile.TileContext(nc) as tc, ExitStack() as ctx:
        # ---------- persistent SBUF pools ----------
        consts = ctx.enter_context(tc.tile_pool(name="consts", bufs=1))
        persist = ctx.enter_context(tc.tile_pool(name="persist", bufs=1))

        ident16 = consts.tile([128, 128], f16, tag="ident16")
        nc.sync.dma_start(ident16[:], ident16_p[:])
        wo_sb = consts.tile([128, D], f32r, tag="wo")
        nc.sync.dma_start(wo_sb[:], wo_p[:])
        ident = consts.tile([128, 128], f32, tag="ident")
        nc.sync.dma_start(ident[:], ident_p[:])
        negones = consts.tile([1, 128], f16, tag="negones")
        nc.sync.dma_start(negones[:], nones_p[:])
        ones_col = consts.tile([128, 1], f16, tag="ones")
        nc.sync.dma_start(ones_col[:], ones_p[:])
        maskT_sb = [consts.tile([128, 1024], f32, tag=f"maskT{p}", name=f"maskT{p}") for p in range(2)]
        mask2_sb = [consts.tile([128, 1024], f32, tag=f"mask2{p}", name=f"mask2{p}") for p in range(2)]
        for p in range(2):
            nc.sync.dma_start(maskT_sb[p][:], maskT_p[p])
            nc.sync.dma_start(mask2_sb[p][:], mask2_p[p])

        # weight hi/lo tiles, SBUF layout [128, (di,dk)]
        def load_w(name, param):
            t = consts.tile([128, 8 * DK], f16, tag=name, name=name)
            nc.sync.dma_start(
                t[:].rearrange("p (di dk) -> p di dk", di=8),
                param[:].rearrange("(di p) dk -> p di dk", p=128))
            return t
        wqh_sb = load_w("wqh", wq_h); wql_sb = load_w("wql", wq_l)
        wkh_sb = load_w("wkh", wk_h); wkl_sb = load_w("wkl", wk_l)
        wvh_sb = load_w("wvh", wv_h); wvl_sb = load_w("wvl", wv_l)

        # persistent activation tiles
        qpTh = persist.tile([128, RPC], f16, tag="qpTh")
        qpTl = persist.tile([128, RPC], f16, tag="qpTl")
        kpTh = persist.tile([128, S], f16, tag="kpTh")
        kpTl = persist.tile([128, S], f16, tag="kpTl")
        vp_sb = [persist.tile([128, 8 * DK], f16, tag=f"vp{g}", name=f"vp{g}") for g in range(8)]

        # ---------- phase A/B: split, transpose, project, gather ----------
        with tc.tile_pool(name="xnat", bufs=3) as xnat, \
             tc.tile_pool(name="xsplit", bufs=4) as xsplit, \
             tc.tile_pool(name="xT", bufs=26) as xTp, \
             tc.tile_pool(name="projps", bufs=2, space="PSUM") as projps, \
             tc.tile_pool(name="tps", bufs=4, space="PSUM") as tps, \
             tc.tile_pool(name="projsb", bufs=4) as projsb:

            def split_tensor(xin, with_l):
                # batched natural loads (256 rows/DMA), fp16 h/l split,
                # PE-transpose [128,128] blocks into [D-sub, rows] tiles.
                xTh = [xTp.tile([128, KPC], f16, tag="xT", name="xT")
                       for _ in range(8)]
                xTl = [xTp.tile([128, KPC], f16, tag="xT", name="xT")
                       for _ in range(8)] if with_l else None
                for i2 in range(4):
                    xt = xnat.tile([128, 2 * D], f32, tag="xnat")
                    nc.sync.dma_start(
                        xt[:].rearrange("p (g d) -> p g d", g=2),
                        xin[256 * i2:256 * (i2 + 1), :]
                        .rearrange("(g p) d -> p g d", p=128))
                    xh = xsplit.tile([128, 2 * D], f16, tag="xh")
                    nc.vector.tensor_copy(xh[:], xt[:])
                    xl = None
                    if with_l:
                        xl = xsplit.tile([128, 2 * D], f16, tag="xl")
                        nc.gpsimd.tensor_sub(xl[:], xt[:], xh[:])
                    rs2 = slice(256 * i2, 256 * (i2 + 1))
                    for di in range(8):
                        pt = tps.tile([128, 256], f16, tag="tps", name="tps")
                        for g in range(2):
                            dsl = slice(1024 * g + 128 * di,
                                        1024 * g + 128 * (di + 1))
                            nc.tensor.transpose(pt[:, 128 * g:128 * (g + 1)],
                                                xh[:, dsl], ident16[:])
                        nc.scalar.copy(xTh[di][:, rs2], pt[:])
                        if with_l:
                            pt2 = tps.tile([128, 256], f16, tag="tps",
                                           name="tps2")
                            for g in range(2):
                                dsl = slice(1024 * g + 128 * di,
                                            1024 * g + 128 * (di + 1))
                                nc.tensor.transpose(
                                    pt2[:, 128 * g:128 * (g + 1)],
                                    xl[:, dsl], ident16[:])
                            nc.vector.tensor_copy(xTl[di][:, rs2], pt2[:])
                return xTh, xTl

            # ---- k ----
            kTh, kTl = split_tensor(k_sh, True)
            for half in range(2):
                ps = projps.tile([128, 512], f32, tag="projps")
                sl = slice(512 * half, 512 * (half + 1))
                n = 0
                for wt, xt in ((wkh_sb, kTh), (wkl_sb, kTh), (wkh_sb, kTl)):
                    for di in range(8):
                        nc.tensor.matmul(ps[:], wt[:, 128 * di:128 * (di + 1)],
                                         xt[di][:, sl], start=(n == 0), stop=(n == 23))
                        n += 1
                hh = projsb.tile([128, 512], f16, tag="ph")
                nc.scalar.copy(hh[:], ps[:])
                ll = projsb.tile([128, 512], f16, tag="pl")
                nc.vector.tensor_sub(ll[:], ps[:], hh[:])
                nc.sync.dma_start(agk_in[0, :, sl], hh[:])
                nc.sync.dma_start(agk_in[1, :, sl], ll[:])
            nc.gpsimd.collective_compute(
                "AllGather", mybir.AluOpType.bypass, replica_groups=rgroups,
                ins=[agk_in[:]], outs=[agk_out[:]])

            # ---- v ----
            vTh, _ = split_tensor(v_sh, False)
            for kt in range(8):
                ps = projps.tile([128, 128], f32, tag="projpsv")
                ksl = slice(128 * kt, 128 * (kt + 1))
                n = 0
                for wt in (wvh_sb, wvl_sb):
                    for di in range(8):
                        nc.tensor.matmul(ps[:], vTh[di][:, ksl],
                                         wt[:, 128 * di:128 * (di + 1)],
                                         start=(n == 0), stop=(n == 15))
                        n += 1
                vh16 = projsb.tile([128, 128], f16, tag="vh16")
                nc.scalar.copy(vh16[:], ps[:])
                nc.sync.dma_start(
                    agv_in[ksl, :].rearrange("p dv -> p dv"), vh16[:])
            nc.gpsimd.collective_compute(
                "AllGather", mybir.AluOpType.bypass, replica_groups=rgroups,
                ins=[agv_in[:]], outs=[agv_out[:]])

            # ---- q ----
            qTh, qTl = split_tensor(q_sh, True)
            for half in range(2):
                ps = projps.tile([128, 512], f32, tag="projps")
                sl = slice(512 * half, 512 * (half + 1))
                n = 0
                for wt, xt in ((wqh_sb, qTh), (wql_sb, qTh), (wqh_sb, qTl)):
                    for di in range(8):
                        nc.tensor.matmul(ps[:], wt[:, 128 * di:128 * (di + 1)],
                                         xt[di][:, sl], start=(n == 0), stop=(n == 23))
                        n += 1
                nc.scalar.copy(qpTh[:, sl], ps[:])
                nc.vector.tensor_sub(qpTl[:, sl], ps[:], qpTh[:, sl])

            # ---- gathered loads ----
            nc.sync.dma_start(
                kpTh[:].rearrange("p (g f) -> p g f", g=8),
                agk_out[:, 0].rearrange("g p f -> p g f"))
            nc.sync.dma_start(
                kpTl[:].rearrange("p (g f) -> p g f", g=8),
                agk_out[:, 1].rearrange("g p f -> p g f"))
            for g in range(8):
                nc.sync.dma_start(
                    vp_sb[g][:].rearrange("p (rb dv) -> p rb dv", rb=8),
                    agv_out[g].rearrange("a b -> (a b)")
                    .rearrange("(rb p dv) -> p rb dv", rb=8, p=128))

        # ---------- phase C: attention ----------
        with tc.tile_pool(name="statps", bufs=2, space="PSUM") as statps, \
             tc.tile_pool(name="sTps", bufs=2, space="PSUM") as sTps, \
             tc.tile_pool(name="oTps", bufs=2, space="PSUM") as oTps, \
             tc.tile_pool(name="smps", bufs=1, space="PSUM") as smps, \
             tc.tile_pool(name="mps", bufs=1, space="PSUM") as mps, \
             tc.tile_pool(name="mpool", bufs=2) as mpool, \
             tc.tile_pool(name="pTpool", bufs=3) as pTpool, \
             tc.tile_pool(name="osb", bufs=4) as osb:

            for oct_ in range(NOCT):
                par = oct_ % 2
                rsl = slice(128 * oct_, 128 * (oct_ + 1))
                nkt = 8 * (oct_ + 1)
                ngrp = 2 * (oct_ + 1)

                # ---- stats: row max m over keys [0, 1024*(oct+1)) ----
                m_b = mpool.tile([128, 1], f32, tag="m")
                for st in range(ngrp):
                    ps_s = statps.tile([128, 512], f32, tag="stat")
                    nc.tensor.matmul(ps_s[:], qpTh[:, rsl],
                                     kpTh[:, 512 * st:512 * (st + 1)],
                                     start=True, stop=True)
                    if st >= 2 * oct_:
                        w = st - 2 * oct_
                        nc.vector.tensor_add(
                            ps_s[:], ps_s[:],
                            mask2_sb[par][:, 512 * w:512 * (w + 1)])
                    mx = mpool.tile([128, 1], f32, tag="mx")
                    nc.vector.reduce_max(mx[:], ps_s[:], axis=mybir.AxisListType.X)
                    if st == 0:
                        nc.vector.tensor_copy(m_b[:], mx[:])
                    else:
                        nc.vector.tensor_max(m_b[:], m_b[:], mx[:])

                # m as a row vector [1,128] fp16 (for the K=1 rank-1 subtract)
                mrep_ps = mps.tile([1, 128], f32, tag="mrep")
                nc.tensor.transpose(mrep_ps[:], m_b[:], ident[:])
                # m/16 in fp16 (m can exceed fp16 max; the rank-1 lhsT is -16)
                mrep = mpool.tile([1, 128], f16, tag="mrepsb")
                nc.scalar.mul(mrep[:], mrep_ps[:], 1.0 / 16.0)

                # ---- main pass over key tiles, groups of 4 ----
                oT = oTps.tile([128, 128], f32, tag="oT")
                sm = smps.tile([128, 1], f32, tag="sm")
                for grp in range(ngrp):
                    ps_g = sTps.tile([128, 512], f32, tag="sT")
                    for t4 in range(4):
                        kt = 4 * grp + t4
                        ksl = slice(128 * kt, 128 * (kt + 1))
                        sl = ps_g[:, 128 * t4:128 * (t4 + 1)]
                        nc.tensor.matmul(sl, kpTh[:, ksl], qpTh[:, rsl],
                                         start=True, stop=False)
                        nc.tensor.matmul(sl, kpTh[:, ksl], qpTl[:, rsl],
                                         start=False, stop=False)
                        nc.tensor.matmul(sl, kpTl[:, ksl], qpTh[:, rsl],
                                         start=False, stop=False)
                        nc.tensor.matmul(sl, negones[:], mrep[:],
                                         start=False, stop=True)
                    if grp >= 2 * oct_:
                        w = grp - 2 * oct_
                        nc.vector.tensor_add(
                            ps_g[:], ps_g[:],
                            maskT_sb[par][:, 512 * w:512 * (w + 1)])
                    pT = pTpool.tile([128, 512], f16, tag="pT")
                    nc.scalar.activation(pT[:], ps_g[:],
                                         mybir.ActivationFunctionType.Exp,
                                         scale=INV_SQRT_DK)
                    for t4 in range(4):
                        kt = 4 * grp + t4
                        g, rb = kt // 8, kt % 8
                        psl = pT[:, 128 * t4:128 * (t4 + 1)]
                        nc.tensor.matmul(oT[:],
                                         vp_sb[g][:, 128 * rb:128 * (rb + 1)], psl,
                                         start=(kt == 0), stop=(kt == nkt - 1))
                        nc.tensor.matmul(sm[:], psl, ones_col[:],
                                         start=(kt == 0), stop=(kt == nkt - 1))

                # ---- normalize + output projection ----
                oT_sb = osb.tile([128, 128], f32r, tag="oTsb")
                nc.vector.tensor_copy(oT_sb[:], oT[:])
                rsum = mpool.tile([128, 1], f32, tag="rsum")
                nc.vector.reciprocal(rsum[:], sm[:])
                out_full = osb.tile([128, 1024], f32, tag="outfull",
                                    name="outfull")
                for half in range(2):
                    sl = slice(512 * half, 512 * (half + 1))
                    ps_o = sTps.tile([128, 512], f32, tag="sT")
                    nc.tensor.matmul(ps_o[:], oT_sb[:], wo_sb[:, sl],
                                     start=True, stop=True)
                    nc.scalar.activation(out_full[:, sl], ps_o[:],
                                         mybir.ActivationFunctionType.Copy,
                                         scale=rsum[:])
                nc.sync.dma_start(o_sh[rsl, :], out_full[:])

    nc.compile()
    return nc


def _host_inputs(q, k, v, w_q, w_k, w_v, w_o):
    """Build per-core input maps (host-side sharding)."""
    f16 = np.float16
    # weight hi/lo splits
    def split(w):
        h = w.astype(f16)
        l = (w - h.astype(np.float32)).astype(f16)
        return h, l
    wq_h, wq_l = split(w_q)
    wk_h, wk_l = split(w_k)
    wv_h, wv_l = split(w_v)

    ident = np.eye(128, dtype=np.float32)
    ident16 = np.eye(128, dtype=f16)
    negones = np.full((1, 128), -16.0, dtype=f16)
    ones_col = np.ones((128, 1), dtype=f16)

    kidx = np.arange(128)[:, None]
    t_f = np.arange(1024)[None, :] // 128
    r_f = np.arange(1024)[None, :] % 128
    ridx = np.arange(128)[:, None]
    kk_f = np.arange(1024)[None, :]

    in_maps = []
    for c in range(N_CORES):
        blocks = [8 * o + (c if o % 2 == 0 else 7 - c) for o in range(NOCT)]
        q_rows = np.concatenate([q[128 * gb:128 * (gb + 1)] for gb in blocks])
        maskT = np.empty((2, 128, 1024), np.float32)
        mask2 = np.empty((2, 128, 1024), np.float32)
        for p, pos in enumerate((c, 7 - c)):
            maskT[p] = np.where(128 * t_f + kidx <= 128 * pos + r_f, 0.0, NEG_BIG)
            mask2[p] = np.where(kk_f <= 128 * pos + ridx, 0.0, NEG_BIG)
        in_maps.append({
            "q_sh": np.ascontiguousarray(q_rows),
            "k_sh": np.ascontiguousarray(k[KPC * c:KPC * (c + 1)]),
            "v_sh": np.ascontiguousarray(v[KPC * c:KPC * (c + 1)]),
            "wq_h": wq_h, "wq_l": wq_l, "wk_h": wk_h, "wk_l": wk_l,
            "wv_h": wv_h, "wv_l": wv_l, "wo": w_o,
            "maskT": maskT, "mask2": mask2, "ident": ident, "ident16": ident16,
            "negones": negones, "ones_col": ones_col,
        })
    return in_maps


def kernel(q, k, v, w_q, w_k, w_v, w_o):
    from concourse.bass_utils import run_bass_kernel_spmd

    q = np.asarray(q, dtype=np.float32)
    k = np.asarray(k, dtype=np.float32)
    v = np.asarray(v, dtype=np.float32)
    w_q = np.asarray(w_q, dtype=np.float32)
    w_k = np.asarray(w_k, dtype=np.float32)
    w_v = np.asarray(w_v, dtype=np.float32)
    w_o = np.asarray(w_o, dtype=np.float32)

    if "nc" not in _CACHE:
        _CACHE["nc"] = _build()
    nc = _CACHE["nc"]

    in_maps = _host_inputs(q, k, v, w_q, w_k, w_v, w_o)
    res = run_bass_kernel_spmd(nc, in_maps, list(range(N_CORES)))

    out = np.empty((S, D), dtype=np.float32)
    for c in range(N_CORES):
        o_sh = res.results[c]["o_sh"]
        for o in range(NOCT):
            gb = 8 * o + (c if o % 2 == 0 else 7 - c)
            out[128 * gb:128 * (gb + 1)] = o_sh[128 * o:128 * (o + 1)]
    return out
